# revision 2
# baseline (speedup 1.0000x reference)
"""CaptioningRNN (LSTM + spatial attention + vocab loss) on 8 Trainium2 cores.

v2 strategy: pure data-parallel over batch (per sharding hint) — ZERO device
collectives. Each core owns 16 captions/images; all weights replicated.
Host sums the 8 per-core partial losses (+ the b_vocab[y] term).

Per-core layouts (n=16 local batch, h=1024 hidden, p=16 spatial, G=4096):
 - AT   [128 hcol, (hc8 n16 p16)]  bf16   A^T mega-tile: A[n, 128*hc+hcol, p]
 - hT   [128 hcol, (hc8 n16)]      bf16   h^T; lhsT K-tile hc = hT[:, 16hc:16hc+16]
 - c    [64 (half2, n16+16garb), 512]  f32  LSTM cell state (col-tiled layout)
 - gate psum: 2 tiles [128, 512] (4x col-tiled via tile_position, M=16/strip)
 - hs_dram [TP, 128, 128] bf16  h_t rows staged to HBM for the vocab GEMM

Per step: scores via DVE mul + 8 ones-matmuls (partition reduce), softmax on
[1, 256], PE K=1 broadcast of softmax weights to 128 partitions, attention via
DVE mul+reduce straight in h^T layout (no per-step transposes for attn),
col-tiled gate matmuls, pointwise LSTM, 4 PE transposes for h^T.

Vocab phase: hs@Wvoc streamed from HBM in 256-col chunks, logsumexp via
ACT-exp accum_out; label logits via DVE mul + ones-matmul; per-core loss
assembled on device, summed on host.

Env knobs: KREP (repeat recurrence+vocab for device-time amplification).
"""
import sys, os

sys.path.insert(0, "/opt/trn_rl_repo")

import numpy as np
import ml_dtypes

import concourse.bass as bass
import concourse.bacc as bacc
import concourse.mybir as mybir
import concourse.tile as tile
from concourse.bass_utils import run_bass_kernel_spmd


BF16 = ml_dtypes.bfloat16
F32 = mybir.dt.float32
BF = mybir.dt.bfloat16

NCORES = 8
N = 128          # global batch
NL = 16          # local batch
CIN = 1280
WD = 512         # embed dim
H = 1024         # hidden
V = 10000        # vocab
P = 16           # spatial positions
G = 4 * H        # gate width
VC = 256         # vocab chunk width
AluOp = mybir.AluOpType
Act = mybir.ActivationFunctionType


def _vchunks():
    out, off = [], 0
    while off < V:
        ln = min(VC, V - off)
        out.append((off, ln))
        off += ln
    return out


def build(T):
    KREP = int(os.environ.get("KREP", "1"))  # timing amplification only
    TG = (T + 7) // 8                        # vocab t-groups
    TP = TG * 8                              # padded t rows
    nc = bacc.Bacc("TRN2", target_bir_lowering=False, debug=False,
                   num_devices=NCORES)

    def din(name, shape, dt):
        return nc.dram_tensor(name, shape, dt, kind="ExternalInput").ap()

    imgsT = din("imgsT", [CIN, NL * P], BF)       # [CIN, (n p)]
    xembT = din("xembT", [WD, T * NL], BF)        # [WD, (t n)]
    wproj = din("wproj", [CIN, H], BF)
    bproj = din("bproj", [1, H], BF)
    wx = din("wx", [WD, G], BF)
    wh = din("wh", [H, G], BF)
    watt = din("watt", [H, G], BF)
    bg = din("bg", [1, G], BF)
    wvoc = din("wvoc", [H, V], BF)
    bvoc = din("bvoc", [1, V], BF)
    wyT = din("wyT", [T, 128, 128], BF)           # [t, hcol, (hc n)]
    mask_f = din("mask_f", [1, T * NL], F32)      # [(t n)]
    ident = din("ident", [128, 128], BF)
    identf = din("identf", [128, 128], F32)
    ones1 = din("ones1", [128, 1], BF)            # value 1
    onesk1 = din("onesk1", [1, 128], BF)          # value 1

    loss_out = nc.dram_tensor("loss", [1, 1], F32, kind="ExternalOutput").ap()

    VCH = _vchunks()
    wvoc_r = wvoc[:].rearrange("(k p) c -> p k c", p=128)

    with tile.TileContext(nc) as tc:
        with (
            tc.tile_pool(name="dram", bufs=1, space="DRAM") as dram,
            tc.tile_pool(name="persist", bufs=1) as pp,
            tc.tile_pool(name="ps", bufs=1, space="PSUM") as ps,
        ):
            # ---------- persistent SBUF ----------
            whs = pp.tile([128, 8, G], BF, name="whs")
            watts = pp.tile([128, 8, G], BF, name="watts")
            wxs = pp.tile([128, 4, G], BF, name="wxs")
            AT = pp.tile([128, 8, NL, P], BF, name="AT")
            AT_sc = pp.tile([128, 8, NL, P], BF, name="AT_sc")
            xemb_sb = pp.tile([128, 4, T * NL], BF, name="xemb_sb")
            c_sb = pp.tile([128, 256], F32, name="c_sb")
            c0_sb = pp.tile([128, 256], F32, name="c0_sb")
            h0T = pp.tile([128, 128], BF, name="h0T")
            ident_sb = pp.tile([128, 128], BF, name="ident_sb")
            identf_sb = pp.tile([128, 128], F32, name="identf_sb")
            ones1_sb = pp.tile([128, 1], BF, name="ones1_sb")
            onesk1_sb = pp.tile([1, 128], BF, name="onesk1_sb")
            mask_sb = pp.tile([1, T * NL], F32, name="mask_sb")
            zc_all = pp.tile([128, TG, len(VCH)], F32, name="zc_all")
            ll_all = pp.tile([1, T, NL], F32, name="ll_all")
            z_all = pp.tile([1, TP, NL], F32, name="z_all")

            hs_dram = dram.tile([TP, 128, 128], BF, name="hs_dram")

            for dst, src in [(ident_sb, ident), (identf_sb, identf),
                             (ones1_sb, ones1), (onesk1_sb, onesk1),
                             (mask_sb, mask_f)]:
                nc.sync.dma_start(dst[:], src[:])
            nc.sync.dma_start(whs[:], wh[:].rearrange("(k p) c -> p k c", p=128))
            nc.sync.dma_start(watts[:],
                              watt[:].rearrange("(k p) c -> p k c", p=128))
            nc.sync.dma_start(wxs[:], wx[:].rearrange("(k p) c -> p k c", p=128))
            nc.sync.dma_start(xemb_sb[:],
                              xembT[:].rearrange("(k p) c -> p k c", p=128))

            # ---------- P1: projection ----------
            with tc.tile_pool(name="setup", bufs=1) as sp:
                imgs_sb = sp.tile([128, 10, NL * P], BF, name="imgs_sb")
                bproj_sb = sp.tile([1, H], BF, name="bproj_sb")
                pa_sb = sp.tile([128, 2, H], BF, name="pa_sb")
                nc.sync.dma_start(imgs_sb[:],
                                  imgsT[:].rearrange("(k p) c -> p k c", p=128))
                nc.sync.dma_start(bproj_sb[:], bproj[:])
                # zero-pad hs_dram rows T..TP
                if TP > T:
                    zpad = sp.tile([128, 128], BF, name="zpad")
                    nc.vector.memset(zpad[:], 0.0)
                    for t in range(T, TP):
                        nc.sync.dma_start(hs_dram[t], zpad[:])
                # PA[(n p)_m, h] = imgs^T @ wproj + bproj  (wproj streamed)
                for m in range(2):
                    pss = []
                    for ch in range(2):
                        ps_pa = ps.tile([128, 512], F32, name=f"ps_pa{m}{ch}",
                                        tag="bigv", bufs=2)
                        pss.append(ps_pa)
                    for kt in range(10):
                        wp_t = sp.tile([128, H], BF, name=f"wp{m}_{kt}",
                                       tag="wp_t", bufs=2)
                        nc.sync.dma_start(
                            wp_t[:],
                            wproj[:].rearrange("(k p) c -> p k c", p=128)[:, kt, :])
                        for ch in range(2):
                            nc.tensor.matmul(pss[ch][:],
                                             imgs_sb[:, kt, bass.ts(m, 128)],
                                             wp_t[:, bass.ts(ch, 512)],
                                             start=(kt == 0), stop=False)
                    for ch in range(2):
                        nc.tensor.matmul(pss[ch][:], onesk1_sb[:],
                                         bproj_sb[:, bass.ts(ch, 512)],
                                         start=False, stop=True)
                        nc.scalar.copy(pa_sb[:, m, bass.ts(ch, 512)], pss[ch][:])
                # AT[hcol, hc, n, p] via 16 transposes
                for m in range(2):
                    for hc in range(8):
                        pst = ps.tile([128, 128], BF, name=f"ps_t{m}_{hc}",
                                      tag="tr", bufs=1)
                        nc.tensor.transpose(pst[:],
                                            pa_sb[:, m, bass.ts(hc, 128)],
                                            ident_sb[:])
                        nc.scalar.copy(
                            AT[:, hc, bass.ts(m, 8), :]
                            .rearrange("q a b -> q (a b)"), pst[:])
                # h0T = mean_p A
                h0T_f = sp.tile([128, 128], F32, name="h0T_f")
                nc.vector.tensor_reduce(
                    h0T_f[:], AT[:].rearrange("q hc n p -> q (hc n) p"),
                    mybir.AxisListType.X, AluOp.add)
                nc.scalar.mul(h0T[:], h0T_f[:], 1.0 / P)
                nc.scalar.mul(AT_sc[:], AT[:], 1.0 / 32.0)
                # c0 = h0 in pointwise layout [128(qq,n), 256]
                # H = 256*qq + 128*rb + hcol, hc = 2*qq + rb
                nc.vector.memset(c0_sb[:], 0.0)
                for hc in range(8):
                    qq, rb = hc // 2, hc % 2
                    ps_c0 = ps.tile([128, 128], BF, name=f"ps_c0{hc}",
                                    tag="small", bufs=1)
                    nc.tensor.transpose(ps_c0[32 * qq:32 * qq + 16, :],
                                        h0T[:, bass.ts(hc, 16)],
                                        ident_sb[:], tile_position=(0, 32 * qq))
                    nc.scalar.copy(
                        c0_sb[32 * qq:32 * qq + 16, bass.ts(rb, 128)],
                        ps_c0[32 * qq:32 * qq + 16, :])

            for rep in range(KREP):
                lp = "" if rep == 0 else f"R{rep}_"
                nc.vector.tensor_copy(c_sb[:], c0_sb[:])

                # ---------- P2: recurrence ----------
                with tc.tile_pool(name=f"rec{lp}", bufs=1) as rp:
                    bg16 = rp.tile([1, G], BF, name=f"bg16{lp}", tag="bg16",
                                   bufs=1)
                    nc.sync.dma_start(bg16[:], bg[:])
                    hT_prev = h0T
                    for t in range(T):
                        # scores: e = (AT/32) * hT; partition-reduce via PE
                        e = rp.tile([128, 8, NL, P], BF, name=f"e{lp}{t}",
                                    tag="e", bufs=1)
                        nc.vector.tensor_mul(
                            e[:], AT_sc[:],
                            hT_prev[:].rearrange("q (hc n) -> q hc n", hc=8)
                            .unsqueeze(3).broadcast_to([128, 8, NL, P]))
                        ps_sc = ps.tile([1, NL * P], F32, name=f"ps_sc{lp}{t}",
                                        tag="small", bufs=1)
                        for hc in range(8):
                            nc.tensor.matmul(
                                ps_sc[:], ones1_sb[:],
                                e[:, hc].rearrange("q n p -> q (n p)"),
                                start=(hc == 0), stop=(hc == 7))
                        # softmax over p (no max-sub: |scores| small)
                        es = rp.tile([1, NL, P], F32, name=f"es{lp}{t}",
                                     tag="es", bufs=1)
                        nc.scalar.activation(
                            es[:].rearrange("o n p -> o (n p)"), ps_sc[:],
                            Act.Exp)
                        zs = rp.tile([1, NL], F32, name=f"zs{lp}{t}", tag="zs",
                                     bufs=1)
                        nc.vector.tensor_reduce(zs[:], es[:],
                                                mybir.AxisListType.X, AluOp.add)
                        rz = rp.tile([1, NL], F32, name=f"rz{lp}{t}", tag="rz",
                                     bufs=1)
                        nc.vector.reciprocal(rz[:], zs[:])
                        w_bf = rp.tile([1, NL, P], BF, name=f"w{lp}{t}",
                                       tag="w_bf", bufs=1)
                        nc.vector.tensor_mul(
                            w_bf[:], es[:],
                            rz[:].unsqueeze(2).broadcast_to([1, NL, P]))
                        # broadcast w to 128 partitions via K=1 matmul
                        ps_wb = ps.tile([128, NL * P], F32, name=f"ps_wb{lp}{t}",
                                        tag="tr", bufs=1)
                        nc.tensor.matmul(ps_wb[:], onesk1_sb[:],
                                         w_bf[:].rearrange("o n p -> o (n p)"),
                                         start=True, stop=True)
                        wb = rp.tile([128, NL, P], BF, name=f"wb{lp}{t}",
                                     tag="wb", bufs=1)
                        nc.scalar.copy(wb[:].rearrange("q n p -> q (n p)"),
                                       ps_wb[:])
                        # attention in h^T layout: attnT = sum_p AT * wb
                        e3 = rp.tile([128, 8, NL, P], BF, name=f"e3{lp}{t}",
                                     tag="e", bufs=1)
                        nc.vector.tensor_mul(
                            e3[:], AT[:],
                            wb[:].unsqueeze(1).broadcast_to([128, 8, NL, P]))
                        attnT_f = rp.tile([128, 8, NL], F32, name=f"af{lp}{t}",
                                          tag="attnT_f", bufs=1)
                        nc.vector.tensor_reduce(attnT_f[:], e3[:],
                                                mybir.AxisListType.X, AluOp.add)
                        attnT = rp.tile([128, 128], BF, name=f"at{lp}{t}",
                                        tag="attnT", bufs=1)
                        nc.scalar.copy(
                            attnT[:], attnT_f[:].rearrange("q hc n -> q (hc n)"))
                        # gate matmuls: one psum per gate (lane-aligned
                        # pointwise), H-quarters as col strips 0/32/64/96.
                        # Pass 1 (bias+x+h) runs while attention DVE computes;
                        # pass 2 (attn) after attnT. qq-innermost emission
                        # interleaves the 4 col groups for PE concurrency.
                        gps = []
                        for q in range(4):
                            gp = ps.tile([128, 256], F32, name=f"g{q}{lp}{t}",
                                         tag=f"g{q}", bufs=1)
                            gps.append(gp)

                        def gsl(q, qq):
                            return gps[q][32 * qq:32 * qq + 16, :]

                        for q in range(4):
                            for qq in range(4):
                                nc.tensor.matmul(
                                    gsl(q, qq), onesk1_sb[:, 0:NL],
                                    bg16[:, 1024 * q + 256 * qq:
                                         1024 * q + 256 * qq + 256],
                                    start=True, stop=False,
                                    tile_position=(0, 32 * qq))
                        for q in range(4):
                            for kt in range(4):
                                for qq in range(4):
                                    co = 1024 * q + 256 * qq
                                    nc.tensor.matmul(
                                        gsl(q, qq),
                                        xemb_sb[:, kt, bass.ts(t, NL)],
                                        wxs[:, kt, co:co + 256],
                                        start=False, stop=False,
                                        tile_position=(0, 32 * qq))
                            for kt in range(8):
                                for qq in range(4):
                                    co = 1024 * q + 256 * qq
                                    nc.tensor.matmul(
                                        gsl(q, qq),
                                        hT_prev[:, bass.ts(kt, 16)],
                                        whs[:, kt, co:co + 256],
                                        start=False, stop=False,
                                        tile_position=(0, 32 * qq))
                        for q in range(4):
                            for kt in range(8):
                                for qq in range(4):
                                    co = 1024 * q + 256 * qq
                                    nc.tensor.matmul(
                                        gsl(q, qq),
                                        attnT[:, bass.ts(kt, 16)],
                                        watts[:, kt, co:co + 256],
                                        start=False, stop=(kt == 7),
                                        tile_position=(0, 32 * qq))
                        # pointwise LSTM, all tiles [128(qq,n), 256]
                        sg_i = rp.tile([128, 256], BF, name=f"sgi{lp}{t}",
                                       tag="sg_i", bufs=1)
                        nc.scalar.activation(sg_i[:], gps[0][:], Act.Sigmoid)
                        sg_f = rp.tile([128, 256], BF, name=f"sgf{lp}{t}",
                                       tag="sg_f", bufs=1)
                        nc.scalar.activation(sg_f[:], gps[1][:], Act.Sigmoid)
                        sg_o = rp.tile([128, 256], BF, name=f"sgo{lp}{t}",
                                       tag="sg_o", bufs=1)
                        nc.scalar.activation(sg_o[:], gps[2][:], Act.Sigmoid)
                        t_g = rp.tile([128, 256], BF, name=f"tg{lp}{t}",
                                      tag="t_g", bufs=1)
                        nc.scalar.activation(t_g[:], gps[3][:], Act.Tanh)
                        ig = rp.tile([128, 256], F32, name=f"ig{lp}{t}",
                                     tag="ig", bufs=1)
                        nc.vector.tensor_mul(ig[:], sg_i[:], t_g[:])
                        nc.vector.tensor_mul(c_sb[:], sg_f[:], c_sb[:])
                        nc.vector.tensor_add(c_sb[:], c_sb[:], ig[:])
                        tc_t = rp.tile([128, 256], BF, name=f"tc{lp}{t}",
                                       tag="tc_t", bufs=1)
                        nc.scalar.activation(tc_t[:], c_sb[:], Act.Tanh)
                        h_sl = rp.tile([128, 256], BF, name=f"h{lp}{t}",
                                       tag="h_sl", bufs=1)
                        nc.vector.tensor_mul(h_sl[:], sg_o[:], tc_t[:])
                        # h^T via 2 PE transposes + 8 strip copies
                        # hc = 2*qq + rb
                        hT_next = rp.tile([128, 128], BF, name=f"hT{lp}{t}",
                                          tag="hT", bufs=2)
                        for rb in range(2):
                            pst = ps.tile([128, 128], BF,
                                          name=f"pst{lp}{t}_{rb}",
                                          tag="tr", bufs=1)
                            nc.tensor.transpose(pst[:],
                                                h_sl[:, bass.ts(rb, 128)],
                                                ident_sb[:])
                            for qq in range(4):
                                hc = 2 * qq + rb
                                nc.scalar.copy(
                                    hT_next[:, bass.ts(hc, 16)],
                                    pst[:, 32 * qq:32 * qq + 16])
                        nc.sync.dma_start(hs_dram[t], zpad[:])
                # PA[(n p)_m, h] = imgs^T @ wproj + bproj  (wproj streamed)
                for m in range(2):
                    pss = []
                    for ch in range(2):
                        ps_pa = ps.tile([128, 512], F32, name=f"ps_pa{m}{ch}",
                                        tag="bigv", bufs=2)
                        pss.append(ps_pa)
                    for kt in range(10):
                        wp_t = sp.tile([128, H], BF, name=f"wp{m}_{kt}",
                                       tag="wp_t", bufs=2)
                        nc.sync.dma_start(
                            wp_t[:],
                            wproj[:].rearrange("(k p) c -> p k c", p=128)[:, kt, :])
                        for ch in range(2):
                            nc.tensor.matmul(pss[ch][:],
                                             imgs_sb[:, kt, bass.ts(m, 128)],
                                             wp_t[:, bass.ts(ch, 512)],
                                             start=(kt == 0), stop=False)
                    for ch in range(2):
                        nc.tensor.matmul(pss[ch][:], onesk1_sb[:],
                                         bproj_sb[:, bass.ts(ch, 512)],
                                         start=False, stop=True)
                        nc.scalar.copy(pa_sb[:, m, bass.ts(ch, 512)], pss[ch][:])
                # AT[hcol, hc, n, p] via 16 transposes
                for m in range(2):
                    for hc in range(8):
                        pst = ps.tile([128, 128], BF, name=f"ps_t{m}_{hc}",
                                      tag="tr", bufs=1)
                        nc.tensor.transpose(pst[:],
                                            pa_sb[:, m, bass.ts(hc, 128)],
                                            ident_sb[:])
                        nc.scalar.copy(
                            AT[:, hc, bass.ts(m, 8), :]
                            .rearrange("q a b -> q (a b)"), pst[:])
                # h0T = mean_p A
                h0T_f = sp.tile([128, 128], F32, name="h0T_f")
                nc.vector.tensor_reduce(
                    h0T_f[:], AT[:].rearrange("q hc n p -> q (hc n) p"),
                    mybir.AxisListType.X, AluOp.add)
                nc.scalar.mul(h0T[:], h0T_f[:], 1.0 / P)
                nc.scalar.mul(AT_sc[:], AT[:], 1.0 / 32.0)
                # c0 = h0 in pointwise layout [128(qq,n), 256]
                # H = 256*qq + 128*rb + hcol, hc = 2*qq + rb
                nc.vector.memset(c0_sb[:], 0.0)
                for hc in range(8):
                    qq, rb = hc // 2, hc % 2
                    ps_c0 = ps.tile([128, 128], BF, name=f"ps_c0{hc}",
                                    tag="small", bufs=1)
                    nc.tensor.transpose(ps_c0[32 * qq:32 * qq + 16, :],
                                        h0T[:, bass.ts(hc, 16)],
                                        ident_sb[:], tile_position=(0, 32 * qq))
                    nc.scalar.copy(
                        c0_sb[32 * qq:32 * qq + 16, bass.ts(rb, 128)],
                        ps_c0[32 * qq:32 * qq + 16, :])

            for rep in range(KREP):
                lp = "" if rep == 0 else f"R{rep}_"
                nc.vector.tensor_copy(c_sb[:], c0_sb[:])

                # ---------- P2: recurrence ----------
                with tc.tile_pool(name=f"rec{lp}", bufs=1) as rp:
                    bg16 = rp.tile([1, G], BF, name=f"bg16{lp}", tag="bg16",
                                   bufs=1)
                    nc.sync.dma_start(bg16[:], bg[:])
                    hT_prev = h0T
                    for t in range(T):
                        # scores: e = (AT/32) * hT; partition-reduce via PE
                        e = rp.tile([128, 8, NL, P], BF, name=f"e{lp}{t}",
                                    tag="e", bufs=1)
                        nc.vector.tensor_mul(
                            e[:], AT_sc[:],
                            hT_prev[:].rearrange("q (hc n) -> q hc n", hc=8)
                            .unsqueeze(3).broadcast_to([128, 8, NL, P]))
                        ps_sc = ps.tile([1, NL * P], F32, name=f"ps_sc{lp}{t}",
                                        tag="small", bufs=1)
                        for hc in range(8):
                            nc.tensor.matmul(
                                ps_sc[:], ones1_sb[:],
                                e[:, hc].rearrange("q n p -> q (n p)"),
                                start=(hc == 0), stop=(hc == 7))
                        # softmax over p (no max-sub: |scores| small)
                        es = rp.tile([1, NL, P], F32, name=f"es{lp}{t}",
                                     tag="es", bufs=1)
                        nc.scalar.activation(
                            es[:].rearrange("o n p -> o (n p)"), ps_sc[:],
                            Act.Exp)
                        zs = rp.tile([1, NL], F32, name=f"zs{lp}{t}", tag="zs",
                                     bufs=1)
                        nc.vector.tensor_reduce(zs[:], es[:],
                                                mybir.AxisListType.X, AluOp.add)
                        rz = rp.tile([1, NL], F32, name=f"rz{lp}{t}", tag="rz",
                                     bufs=1)
                        nc.vector.reciprocal(rz[:], zs[:])
                        w_bf = rp.tile([1, NL, P], BF, name=f"w{lp}{t}",
                                       tag="w_bf", bufs=1)
                        nc.vector.tensor_mul(
                            w_bf[:], es[:],
                            rz[:].unsqueeze(2).broadcast_to([1, NL, P]))
                        # broadcast w to 128 partitions via K=1 matmul
                        ps_wb = ps.tile([128, NL * P], F32, name=f"ps_wb{lp}{t}",
                                        tag="tr", bufs=1)
                        nc.tensor.matmul(ps_wb[:], onesk1_sb[:],
                                         w_bf[:].rearrange("o n p -> o (n p)"),
                                         start=True, stop=True)
                        wb = rp.tile([128, NL, P], BF, name=f"wb{lp}{t}",
                                     tag="wb", bufs=1)
                        nc.scalar.copy(wb[:].rearrange("q n p -> q (n p)"),
                                       ps_wb[:])
                        # attention in h^T layout: attnT = sum_p AT * wb
                        e3 = rp.tile([128, 8, NL, P], BF, name=f"e3{lp}{t}",
                                     tag="e", bufs=1)
                        nc.vector.tensor_mul(
                            e3[:], AT[:],
                            wb[:].unsqueeze(1).broadcast_to([128, 8, NL, P]))
                        attnT_f = rp.tile([128, 8, NL], F32, name=f"af{lp}{t}",
                                          tag="attnT_f", bufs=1)
                        nc.vector.tensor_reduce(attnT_f[:], e3[:],
                                                mybir.AxisListType.X, AluOp.add)
                        attnT = rp.tile([128, 128], BF, name=f"at{lp}{t}",
                                        tag="attnT", bufs=1)
                        nc.scalar.copy(
                            attnT[:], attnT_f[:].rearrange("q hc n -> q (hc n)"))
                        # gate matmuls: one psum per gate (lane-aligned
                        # pointwise), H-quarters as col strips (0,32,64,96)
                        gps = []
                        for q in range(4):
                            gp = ps.tile([128, 256], F32, name=f"g{q}{lp}{t}",
                                         tag=f"g{q}", bufs=1)
                            gps.append(gp)
                        for q in range(4):
                            for qq in range(4):
                                sl = gps[q][32 * qq:32 * qq + 16, :]
                                tp = (0, 32 * qq)
                                co = 1024 * q + 256 * qq
                                bg_t = rp.tile([1, 256], BF,
                                               name=f"bg{lp}{t}_{q}{qq}",
                                               tag="bg_t", bufs=2)
                                nc.sync.dma_start(bg_t[:], bg[:, co:co + 256])
                                for kt in range(4):
                                    nc.tensor.matmul(
                                        sl, xemb_sb[:, kt, bass.ts(t, NL)],
                                        wxs[:, kt, co:co + 256],
                                        start=(kt == 0), stop=False,
                                        tile_position=tp)
                                for kt in range(8):
                                    nc.tensor.matmul(
                                        sl, hT_prev[:, bass.ts(kt, 16)],
                                        whs[:, kt, co:co + 256],
                                        start=False, stop=False,
                                        tile_position=tp)
                                for kt in range(8):
                                    nc.tensor.matmul(
                                        sl, attnT[:, bass.ts(kt, 16)],
                                        watts[:, kt, co:co + 256],
                                        start=False, stop=False,
                                        tile_position=tp)
                                nc.tensor.matmul(
                                    sl, onesk1_sb[:, 0:NL], bg_t[:],
                                    start=False, stop=True, tile_position=tp)
                        # pointwise LSTM, all tiles [128(qq,n), 256]
                        sg_i = rp.tile([128, 256], BF, name=f"sgi{lp}{t}",
                                       tag="sg_i", bufs=1)
                        nc.scalar.activation(sg_i[:], gps[0][:], Act.Sigmoid)
                        sg_f = rp.tile([128, 256], BF, name=f"sgf{lp}{t}",
                                       tag="sg_f", bufs=1)
                        nc.scalar.activation(sg_f[:], gps[1][:], Act.Sigmoid)
                        sg_o = rp.tile([128, 256], BF, name=f"sgo{lp}{t}",
                                       tag="sg_o", bufs=1)
                        nc.scalar.activation(sg_o[:], gps[2][:], Act.Sigmoid)
                        t_g = rp.tile([128, 256], BF, name=f"tg{lp}{t}",
                                      tag="t_g", bufs=1)
                        nc.scalar.activation(t_g[:], gps[3][:], Act.Tanh)
                        ig = rp.tile([128, 256], F32, name=f"ig{lp}{t}",
                                     tag="ig", bufs=1)
                        nc.vector.tensor_mul(ig[:], sg_i[:], t_g[:])
                        nc.vector.tensor_mul(c_sb[:], sg_f[:], c_sb[:])
                        nc.vector.tensor_add(c_sb[:], c_sb[:], ig[:])
                        tc_t = rp.tile([128, 256], BF, name=f"tc{lp}{t}",
                                       tag="tc_t", bufs=1)
                        nc.scalar.activation(tc_t[:], c_sb[:], Act.Tanh)
                        h_sl = rp.tile([128, 256], BF, name=f"h{lp}{t}",
                                       tag="h_sl", bufs=1)
                        nc.vector.tensor_mul(h_sl[:], sg_o[:], tc_t[:])
                        # h^T via 4 PE transposes + 8 strip copies
                        hT_next = rp.tile([128, 128], BF, name=f"hT{lp}{t}",
                                          tag="hT", bufs=2)
                        for tb in range(4):
                            pst = ps.tile([128, 128], BF,
                                          name=f"pst{lp}{t}_{tb}",
                                          tag="tr", bufs=2)
                            nc.tensor.transpose(pst[:, 0:64],
                                                h_sl[:, bass.ts(tb, 128)],
                                                ident_sb[0:64, 0:64])
                            for half in range(2):
                                kt = 4 * half + tb
                                nc.scalar.copy(
                                    hT_next[:, bass.ts(kt, 16)],
                                    pst[:, 32 * half:32 * half + 16])
                        nc.sync.dma_start(hs_dram[t], hT_next[:])
                        hT_prev = hT_next

                # ---------- P3: vocab logsumexp + label logits ----------
                # ci outer / g inner: Wvoc streamed from HBM exactly once.
                with tc.tile_pool(name=f"voc{lp}", bufs=1) as vp:
                    hsgs = []
                    for g in range(TG):
                        # hsg[hcol, hc(kt), t, n]: K-tile slices contiguous
                        hsg = vp.tile([128, 8, 8, NL], BF, name=f"hsg{lp}{g}",
                                      tag=f"hsg{g}", bufs=1)
                        nc.sync.dma_start(
                            hsg[:],
                            hs_dram[8 * g:8 * g + 8]
                            .rearrange("t p (hc n) -> p hc t n", hc=8))
                        hsgs.append(hsg)
                    for ci, (off, ln) in enumerate(VCH):
                        wv_t = vp.tile([128, 8, VC], BF, name=f"wv{lp}{ci}",
                                       tag="wv", bufs=3)
                        bv_t = vp.tile([1, VC], BF, name=f"bv{lp}{ci}",
                                       tag="bv", bufs=2)
                        nc.sync.dma_start(wv_t[:, :, 0:ln],
                                          wvoc_r[:, :, off:off + ln])
                        nc.sync.dma_start(bv_t[:, 0:ln], bvoc[:, off:off + ln])
                        for g in range(TG):
                            ps_v = ps.tile([128, VC], F32,
                                           name=f"psv{lp}{g}_{ci}",
                                           tag="bigv", bufs=2)
                            for kt in range(8):
                                nc.tensor.matmul(
                                    ps_v[:, 0:ln],
                                    hsgs[g][:, kt].rearrange("q a b -> q (a b)"),
                                    wv_t[:, kt, 0:ln],
                                    start=(kt == 0), stop=False)
                            nc.tensor.matmul(ps_v[:, 0:ln], onesk1_sb[:],
                                             bv_t[:, 0:ln],
                                             start=False, stop=True)
                            ex_scr = vp.tile([128, VC], BF,
                                             name=f"ex{lp}{g}_{ci}",
                                             tag="ex", bufs=2)
                            nc.scalar.activation(
                                ex_scr[:, 0:ln], ps_v[:, 0:ln], Act.Exp,
                                accum_out=zc_all[:, g, ci:ci + 1])
                    # label logits: ll[t, n] = hs_t . wy_t
                    for g in range(TG):
                        nt = min(8, T - 8 * g)
                        for tt in range(nt):
                            t = 8 * g + tt
                            wyt_sb = vp.tile([128, 128], BF, name=f"wy{lp}{t}",
                                             tag="wyt", bufs=2)
                            nc.sync.dma_start(wyt_sb[:], wyT[t])
                            ey = vp.tile([128, 128], BF, name=f"ey{lp}{t}",
                                         tag="ey", bufs=2)
                            nc.vector.tensor_mul(
                                ey[:].rearrange("q (a b) -> q a b", a=8),
                                hsgs[g][:, :, tt, :],
                                wyt_sb[:].rearrange("q (a b) -> q a b", a=8))
                            ps_ll = ps.tile([1, 128], F32, name=f"psll{lp}{t}",
                                            tag="small", bufs=1)
                            nc.tensor.matmul(ps_ll[:], ones1_sb[:], ey[:],
                                             start=True, stop=True)
                            nc.vector.tensor_reduce(
                                ll_all[:, t, :],
                                ps_ll[:].rearrange("o (hc n) -> o n hc", hc=8),
                                mybir.AxisListType.X, AluOp.add)

                    # ---------- P4: loss ----------
                    zsum = vp.tile([128, TG], F32, name=f"zsum{lp}",
                                   tag="zsum", bufs=1)
                    nc.vector.tensor_reduce(zsum[:], zc_all[:],
                                            mybir.AxisListType.X, AluOp.add)
                    lnZ = vp.tile([128, TG], F32, name=f"lnZ{lp}", tag="lnZ",
                                  bufs=1)
                    nc.scalar.activation(lnZ[:], zsum[:], Act.Ln)
                    z_scr = dram.tile([128, TG], F32, name=f"z_scr{lp}")
                    nc.sync.dma_start(z_scr[:], lnZ[:])
                    nc.sync.dma_start(
                        z_all[:].rearrange("o (g a) n -> o g a n", g=TG),
                        z_scr[:].rearrange("(a n) g -> g a n", n=NL)
                        .unsqueeze(0))
                    d = vp.tile([1, T * NL], F32, name=f"d{lp}", tag="d",
                                bufs=1)
                    nc.vector.tensor_sub(
                        d[:], z_all[:, 0:T, :].rearrange("o a b -> o (a b)"),
                        ll_all[:].rearrange("o a b -> o (a b)"))
                    nc.vector.tensor_mul(d[:], d[:], mask_sb[:])
                    tot = vp.tile([1, 1], F32, name=f"tot{lp}", tag="tot",
                                  bufs=1)
                    nc.vector.tensor_reduce(tot[:], d[:], mybir.AxisListType.X,
                                            AluOp.add)
                    loss_sb = vp.tile([1, 1], F32, name=f"loss{lp}",
                                      tag="loss_sb", bufs=1)
                    nc.scalar.mul(loss_sb[:], tot[:], 1.0 / N)
                    if rep == KREP - 1:
                        nc.sync.dma_start(loss_out[:], loss_sb[:])

    nc.compile()
    return nc


def host_prep(inputs, T):
    """Build the 8 per-core input maps (numpy); weights shared by reference."""
    g = {k: np.asarray(v) for k, v in inputs.items()}
    images, captions = g["images"], g["captions"]
    W_embed, W_proj, b_proj = g["W_embed"], g["W_proj"], g["b_proj"]
    Wx, Wh, Wattn, b = g["Wx"], g["Wh"], g["Wattn"], g["b"]
    W_vocab, b_vocab = g["W_vocab"], g["b_vocab"]

    cap = np.asarray(captions)
    cap_in = cap[:, :T]
    cap_out = cap[:, 1:T + 1]
    x_emb = W_embed[cap_in]                      # [N, T, WD]
    mask = (cap_out != 0)                        # [N, T]

    shared = {
        "wproj": np.ascontiguousarray(W_proj).astype(BF16),
        "bproj": np.ascontiguousarray(b_proj[None, :]).astype(BF16),
        "wx": np.ascontiguousarray(Wx).astype(BF16),
        "wh": np.ascontiguousarray(Wh).astype(BF16),
        "watt": np.ascontiguousarray(Wattn).astype(BF16),
        "bg": np.ascontiguousarray(b[None, :]).astype(BF16),
        "wvoc": np.ascontiguousarray(W_vocab).astype(BF16),
        "bvoc": np.ascontiguousarray(b_vocab[None, :]).astype(BF16),
        "ident": np.eye(128, dtype=BF16),
        "identf": np.eye(128, dtype=np.float32),
        "ones1": np.ones((128, 1), dtype=BF16),
        "onesk1": np.ones((1, 128), dtype=BF16),
    }

    in_maps = []
    for c in range(NCORES):
        nsl = slice(NL * c, NL * (c + 1))
        imgsT = np.ascontiguousarray(
            images[nsl].reshape(NL, CIN, P).transpose(1, 0, 2)
            .reshape(CIN, NL * P)).astype(BF16)
        xembT = np.ascontiguousarray(
            x_emb[nsl].transpose(2, 1, 0).reshape(WD, T * NL)).astype(BF16)
        # wyT[t, hcol, (hc n)] = W_vocab[128*hc+hcol, cap_out[n, t]]
        wy = W_vocab[:, cap_out[nsl]]            # [H, NL, T]
        wyT = np.ascontiguousarray(
            wy.reshape(8, 128, NL, T).transpose(3, 1, 0, 2)
            .reshape(T, 128, 128)).astype(BF16)
        mask_c = np.ascontiguousarray(
            mask[nsl].T.reshape(1, T * NL)).astype(np.float32)
        in_maps.append({
            "imgsT": imgsT, "xembT": xembT, "wyT": wyT, "mask_f": mask_c,
            **shared,
        })
    host_by = float(np.sum(mask.astype(np.float64) *
                           np.asarray(b_vocab, np.float64)[cap_out]) / N)
    return in_maps, host_by


_CACHE = {}


def _get_built(T):
    if T not in _CACHE:
        _CACHE[T] = build(T)
    return _CACHE[T]


def run(inputs, T=30):
    nc = _get_built(T)
    in_maps, host_by = host_prep(inputs, T)
    res = run_bass_kernel_spmd(nc, in_maps, core_ids=list(range(NCORES)))
    dev = sum(float(res.results[c]["loss"][0, 0]) for c in range(NCORES))
    return np.float32(dev + host_by)


def kernel(**inputs) -> np.ndarray:
    return run(inputs, T=30)
